# revision 27
# baseline (speedup 1.0000x reference)
"""Binarized 3x3 conv (stride 1, pad 1) + bias on 8 Trainium2 NeuronCores.

Full problem: x[32,256,56,56] f32, weight[256,256,3,3] f32, bias[256] f32
-> y[32,256,56,56] f32 with y = conv2d(sign(x), sign(weight), pad=1) + bias
(sign(t) = +1 for t >= 0 else -1).

Sharding: data-parallel over batch. Each of the 8 cores gets 4 images and a
replicated copy of weight/bias, computes its shard fully on-device, and the
host concatenates the 8 output shards.

Per-core kernel:
  - binarize x and w to +/-0.5 with one fused DVE op each ((v>=0) - 0.5);
    the final PSUM->SBUF copy applies scale=4 to undo the 0.25 product
    scale, so results are exactly the +/-1 conv (all integers, exact in f32).
  - x lives zero-padded in SBUF as [128(ci_p), 2(ci_blk), 3376] fp8 per
    image: 58x58 padded image rows + 1 guard element front/back. Pad zeros
    are written once per buffer; data rows are rewritten per image.
  - weight is binarized to bf16, transposed on the PE (36 x 128x128
    transposes via identity), and stored as fp8 lhsT
    [128(ci_p), 2(ci_blk), 9(tap), 256(co)].
  - conv: for each (co_blk, image, 8-row output chunk): accumulate 9
    DoubleRow fp8 matmuls (one per tap, K=256 packed as [128,2]),
    M=128/N=464, into one PSUM bank. Outputs computed at the 2 pad columns
    of each row are garbage and are skipped on the way out.
  - PSUM -> SBUF via ScalarE: Identity(psum*4 + bias[co]), then DMA to y.
"""

import numpy as np

import concourse.bacc as bacc
import concourse.mybir as mybir
import concourse.tile as tile
from concourse.bass_utils import run_bass_kernel_spmd
from concourse.masks import make_identity

F32 = mybir.dt.float32
BF16 = mybir.dt.bfloat16
FP8 = mybir.dt.float8e4
AF = mybir.ActivationFunctionType
ALU = mybir.AluOpType
DR = mybir.MatmulPerfMode.DoubleRow

N_CORES = 8
H = W = 56
WP = 58            # padded row width
CIN = 256
COUT = 256
CI_BLKS = 2        # 256 ci = 2 x 128 partitions
CO_BLKS = 2
R = 8              # output rows per chunk
NCHUNK = H // R    # 7
NV = R * WP        # 464 matmul moving free size
IMG_FA = 3376      # aligned per-ci_blk padded image elems (58*58+2 -> 3376)
WP2 = 57           # shared-pad row width: [pad][56 data], right pad = next row's col 0
NV2 = 455          # 7*57+55+1 moving positions per 8-row chunk
FA2 = 3312         # 57*58 rows + 1 guard, 16-aligned


def _build_conv(tc, y_ap, x_ap, w_ap, b_ap, n_imgs):
    nc = tc.nc
    scale = 4.0  # undo (+/-0.5)*(+/-0.5) = +/-0.25 product scale

    with (
        tc.tile_pool(name="consts", bufs=1) as consts,
        tc.tile_pool(name="wstage", bufs=1) as wstage_pool,
        tc.tile_pool(name="lhst", bufs=1) as lhst_pool,
        tc.tile_pool(name="xstage", bufs=2) as xstage_pool,
        tc.tile_pool(name="xpad", bufs=1) as xpad_pool,
        tc.tile_pool(name="outsb", bufs=3) as out_pool,
        tc.tile_pool(name="tpsum", bufs=2, space="PSUM") as tpsum_pool,
        tc.tile_pool(name="cpsum", bufs=4, space="PSUM") as cpsum_pool,
    ):
        # --- constants -----------------------------------------------------
        # fp8-DR junk matmuls on zeros from t~0: the HAM power governor grants
        # full PE clock only after ~10us of sustained utilization, so start
        # the clock before any DMA lands (bf16 junk would trigger a payback
        # half-clock window mid-conv; the fp8-DR stream does not).
        junk = consts.tile([128, 2, 512], FP8, name="junk")
        nc.vector.memset(junk, 0.0)
        junk_lhs = consts.tile([128, 2, 128], FP8, name="junk_lhs")
        nc.vector.memset(junk_lhs, 0.0)
        ident = consts.tile([128, 128], BF16)
        make_identity(nc, ident)

        def junk_mm():
            jps = cpsum_pool.tile([128, 512], F32, name="ps", tag="ps")
            nc.tensor.matmul(jps, junk_lhs, junk, start=True, stop=True,
                             perf_mode=DR)

        for _ in range(18):
            junk_mm()

        # --- DMA issue order is bandwidth-critical: the conv stream can't
        # start until W_c0 + the first x rows are in SBUF (~360 GB/s/core).
        wstage = wstage_pool.tile([128, CO_BLKS, CIN, 9], F32)
        wb = wstage_pool.tile([128, CO_BLKS, CIN, 9], BF16)
        lhst = lhst_pool.tile([128, CI_BLKS, 9, COUT], FP8)
        xstage0 = xstage_pool.tile([128, CI_BLKS, H * W], F32,
                                   name="xstage0", tag="xstage")

        def dma_w(c, b):
            # one quarter of the weights: co block c, ci block b
            nc.sync.dma_start(
                out=wstage[:, c, b * 128:(b + 1) * 128],
                in_=w_ap[c * 128:(c + 1) * 128, b * 128:(b + 1) * 128].rearrange(
                    "co ci kh kw -> co ci (kh kw)"),
            )

        def dma_x(xstage, n, r0, r1, b):
            nc.sync.dma_start(
                out=xstage[:, b, r0 * W:r1 * W],
                in_=x_ap[n, b * 128:(b + 1) * 128, r0:r1]
                    .rearrange("c h w -> c (h w)"),
            )

        # interleave so the bytes gating the first conv chunk arrive first:
        # lhsT(c=0) needs both W_c0 quarters; chunk k=0 needs x rows 0-8 only
        dma_w(0, 0)
        dma_w(0, 1)
        dma_x(xstage0, 0, 0, 9, 0)
        dma_x(xstage0, 0, 0, 9, 1)
        dma_w(1, 0)
        dma_w(1, 1)
        dma_x(xstage0, 0, 9, 18, 0)
        dma_x(xstage0, 0, 9, 18, 1)
        dma_x(xstage0, 0, 18, 28, 0)
        dma_x(xstage0, 0, 18, 28, 1)
        dma_x(xstage0, 0, 28, 42, 0)
        dma_x(xstage0, 0, 28, 42, 1)
        dma_x(xstage0, 0, 42, H, 0)
        dma_x(xstage0, 0, 42, H, 1)
        bias_sb = consts.tile([128, CO_BLKS], F32)
        nc.scalar.dma_start(out=bias_sb, in_=b_ap.rearrange("(b p) -> p b", p=128))

        # --- weight prep (PSUM->SBUF casts on ScalarE); DVE work emitted in
        # DMA-arrival order ---------------------------------------------------
        # lhsT[ci_p, ci_blk, tap, co] in fp8 (cast on the PSUM->SBUF copy)
        def binz(dst, src):
            nc.vector.tensor_scalar(dst, src, 0.0, 0.5, ALU.is_ge, ALU.subtract)

        def wprep(c, b):
            binz(wb[:, c, b * 128:(b + 1) * 128], wstage[:, c, b * 128:(b + 1) * 128])
            for t in range(9):
                tp = tpsum_pool.tile([128, 128], BF16)
                nc.tensor.transpose(tp, wb[:, c, b * 128:(b + 1) * 128, t], ident)
                nc.scalar.copy(out=lhst[:, b, t, c * 128:(c + 1) * 128],
                               in_=tp)

        # --- x buffers: persistent padded buffers, pad zeros written once
        NXPAD = 3
        xpads = [xpad_pool.tile([128, CI_BLKS, FA2], FP8,
                                name=f"xpad{i}", tag=f"xpad{i}")
                 for i in range(NXPAD)]
        for xp in xpads:
            for b in range(CI_BLKS):
                # top pad row q=0 + left pad of row q=1
                nc.vector.memset(xp[:, b, 0:58], 0.0)
                # bottom pad row q=57 + tail guard/align
                nc.vector.memset(xp[:, b, 57 * 57:FA2], 0.0)
                # left-pad col (elem 57q) of rows q=1..56; doubles as the
                # previous row's right pad in the shared-pad layout
                nc.vector.memset(
                    xp[:, b, 57:57 + 56 * WP2].rearrange(
                        "p (h w) -> p h w", w=WP2)[:, :, 0:1],
                    0.0,
                )

        # --- per-image pipeline -------------------------------------------
        def binz_x(xstage, xpad, r0, r1, b):
            # data row h -> elems 58+57h .. 58+57h+55
            dst = xpad[:, b, 58:58 + H * WP2].rearrange(
                "p (h w) -> p h w", w=WP2)[:, r0:r1, 0:W]
            src = xstage[:, b].rearrange("p (h w) -> p h w", w=W)[:, r0:r1]
            binz(dst, src)

        def conv_chunk(n, xpad, c, k):
            ps = cpsum_pool.tile([128, 456], F32, name="ps", tag="ps")
            for t in range(9):
                kh, kw = divmod(t, 3)
                base = (R * k + kh) * WP2 + kw
                nc.tensor.matmul(
                    ps[:, 0:NV2],
                    lhst[:, 0:2, t, c * 128:(c + 1) * 128],
                    xpad[:, 0:2, base:base + NV2],
                    start=(t == 0),
                    stop=(t == 8),
                    perf_mode=DR,
                )
            osb = out_pool.tile([128, R * W], F32, name="osb")
            nc.scalar.activation(
                out=osb.rearrange("p (r w) -> p r w", w=W),
                in_=ps.rearrange("p (r w) -> p r w", w=WP2)[:, :, 0:56],
                func=AF.Identity,
                bias=bias_sb[:, c:c + 1],
                scale=scale,
            )
            nc.sync.dma_start(
                out=y_ap[n, c * 128:(c + 1) * 128]
                    .rearrange("co h w -> co (h w)")[:, R * W * k:R * W * (k + 1)],
                in_=osb,
            )

        def load_image(n):
            # loads + binarizes image n into its xpad buffer
            xstage = xstage_pool.tile([128, CI_BLKS, H * W], F32,
                                      name=f"xstage{n}", tag="xstage")
            xpad = xpads[n % NXPAD]
            for r0, r1 in ((0, 28), (28, H)):
                for b in range(CI_BLKS):
                    dma_x(xstage, n, r0, r1, b)
                    binz_x(xstage, xpad, r0, r1, b)

        for n in range(n_imgs):
            xpad = xpads[n % NXPAD]
            if n == 0:
                # emission order = engine program order (DMA-arrival order)
                wprep(0, 0)
                wprep(0, 1)
                binz_x(xstage0, xpad, 0, 9, 0)
                binz_x(xstage0, xpad, 0, 9, 1)
                wprep(1, 0)
                wprep(1, 1)
                binz_x(xstage0, xpad, 9, 18, 0)
                binz_x(xstage0, xpad, 9, 18, 1)
                binz_x(xstage0, xpad, 18, 28, 0)
                binz_x(xstage0, xpad, 18, 28, 1)
                binz_x(xstage0, xpad, 28, 42, 0)
                binz_x(xstage0, xpad, 28, 42, 1)
                binz_x(xstage0, xpad, 42, H, 0)
                binz_x(xstage0, xpad, 42, H, 1)
                # no HAM warm-up: sustained fp8-DR conv never triggers the
                # power governor's payback throttle (bf16 junk did), and
                # starting conv early at the cold half-clock beats burning
                # ~8us of junk matmuls first.
            # prefetch image n+1 before image n's conv chunks so its input
            # DMAs take queue priority over image n's output-DMA burst
            if n + 1 < n_imgs:
                load_image(n + 1)
            if n == 0:
                # k-major: image 0's rows are still streaming in from HBM;
                # visiting each row chunk twice (both co blocks) halves the
                # row-consumption rate so the PE never starves behind the DMA
                for k in range(NCHUNK):
                    for c in range(CO_BLKS):
                        conv_chunk(n, xpad, c, k)
            else:
                for c in range(CO_BLKS):
                    for k in range(NCHUNK):
                        conv_chunk(n, xpad, c, k)


ALU = mybir.AluOpType

# ---------------------------------------------------------------------------
# Winograd F(2,3) along H, direct along W.
#
# out rows (2i, 2i+1) from 4 taps: M_t[co, i, p] = sum_{ci,kw} T[t,kw] * V_t,
# with V built from binarized padded rows q=2i..2i+3 (b = sign(x)/2):
#   V0 = b[2i]-b[2i+2], V1 = b[2i+1]+b[2i+2], V2 = b[2i+2]-b[2i+1],
#   V3n = b[2i+3]-b[2i+1]   (tap 3 negated so all combines are adds)
# T0 = g0, T1 = (g0+g1+g2)/2, T2 = T1-g1, T3 = g2  (g = sign(w)/2, per kw)
# out_even = 4*(M0+M1+M2)+bias, out_odd = 4*(M1-M2+M3n)+bias.
# PSUM pairs: P-bank [M1,M2], Q-bank [M0,M3n]; one batched ACT per bank pair
# (scale 4, bias only on Q) -> bf16; DVE: u=P0+P1, v=P0-P1 (bf16 2x), then
# out_even = u+Q0, out_odd = v+Q1 into the f32 out tile.
# ---------------------------------------------------------------------------
NT = 28          # row tiles per image
VROW = 58        # V cols per tile row (incl 2 pads)
VLEN = 29 * VROW     # 28 tiles + 1 guard tile
VPAD = 1696          # 16-aligned tap stride for the DR moving AP
NW = 232             # matmul moving size: 4 tiles x 58


def _build_wino(tc, y_ap, x_ap, w_ap, b_ap, n_imgs):
    nc = tc.nc

    with (
        tc.tile_pool(name="consts", bufs=1) as consts,
        tc.tile_pool(name="wstage", bufs=2) as wstage_pool,
        tc.tile_pool(name="wtap", bufs=4) as wtap_pool,
        tc.tile_pool(name="lhst", bufs=1) as lhst_pool,
        tc.tile_pool(name="xstage", bufs=3) as xstage_pool,
        tc.tile_pool(name="bpad", bufs=1) as bpad_pool,
        tc.tile_pool(name="vfp", bufs=1) as vfp_pool,
        tc.tile_pool(name="psb", bufs=3) as p_pool,
        tc.tile_pool(name="qsb", bufs=3) as q_pool,
        tc.tile_pool(name="uvsb", bufs=3) as uv_pool,
        tc.tile_pool(name="outsb", bufs=3) as out_pool,
        tc.tile_pool(name="pps", bufs=2, space="PSUM") as pps_pool,
        tc.tile_pool(name="qps", bufs=2, space="PSUM") as qps_pool,
    ):
        # --- constants / junk HAM warm-up ---------------------------------
        junk = consts.tile([128, 2, 512], FP8, name="junk")
        nc.vector.memset(junk, 0.0)
        junk_lhs = consts.tile([128, 2, 128], FP8, name="junk_lhs")
        nc.vector.memset(junk_lhs, 0.0)
        ident = consts.tile([128, 128], BF16)
        make_identity(nc, ident)

        def junk_mm():
            jps = qps_pool.tile([128, 2, 512], F32, name="jps", tag="qps")
            nc.tensor.matmul(jps[:, 0], junk_lhs, junk, start=True,
                             stop=True, perf_mode=DR)

        for _ in range(12):
            junk_mm()

        # --- DMA issue order: W c0 first, then img0 rows, W c1 ------------
        wstage = wstage_pool.tile([128, CIN, 9], F32, name="wstage",
                                  tag="wstage")
        wb = wstage_pool.tile([128, CO_BLKS, CIN, 9], BF16)
        lhst = lhst_pool.tile([128, CI_BLKS, 12, COUT], FP8)

        def dma_w(c, b, ws):
            nc.sync.dma_start(
                out=ws[:, b * 128:(b + 1) * 128],
                in_=w_ap[c * 128:(c + 1) * 128, b * 128:(b + 1) * 128]
                    .rearrange("co ci kh kw -> co ci (kh kw)"),
            )

        XR = ((0, 14), (14, 28), (28, 42), (42, H))  # DMA/binz row ranges

        def dma_x(xs, n, ri, b):
            r0, r1 = XR[ri]
            nc.sync.dma_start(
                out=xs[:, (r0 % 28) * W:(r0 % 28 + (r1 - r0)) * W],
                in_=x_ap[n, b * 128:(b + 1) * 128, r0:r1]
                    .rearrange("c h w -> c (h w)"),
            )

        def binz(dst, src):
            nc.vector.tensor_scalar(dst, src, 0.0, 0.5, ALU.is_ge,
                                    ALU.subtract)

        # --- weight prep ---------------------------------------------------
        def wprep(c, ws):
            # binarize this co-block's weights to +-0.5 bf16
            binz(wb[:, c], ws)
            for b in range(CI_BLKS):
                wt = wtap_pool.tile([128, 3, 2, 128], BF16,
                                    name=f"wt{c}{b}", tag=f"wt{c}{b}")
                ci = slice(b * 128, (b + 1) * 128)
                for kw in range(3):
                    g0 = wb[:, c, ci, 0 * 3 + kw]
                    g1 = wb[:, c, ci, 1 * 3 + kw]
                    g2 = wb[:, c, ci, 2 * 3 + kw]
                    u = wt[:, kw, 0]
                    nc.vector.tensor_tensor(u, g0, g2, ALU.add)
                    nc.vector.tensor_tensor(u, u, g1, ALU.add)
                    nc.vector.tensor_scalar_mul(wt[:, kw, 0], u, 0.5)  # T1
                    nc.vector.tensor_tensor(wt[:, kw, 1], wt[:, kw, 0], g1,
                                            ALU.subtract)              # T2
                # transpose 12 taps (t*3+kw) into lhst[ci_p, b, tap, co]
                srcs = []
                for t in range(4):
                    for kw in range(3):
                        if t == 0:
                            srcs.append(wb[:, c, ci, kw])
                        elif t == 1:
                            srcs.append(wt[:, kw, 0])
                        elif t == 2:
                            srcs.append(wt[:, kw, 1])
                        else:
                            srcs.append(wb[:, c, ci, 6 + kw])
                for j0, jn in ((0, 8), (8, 4)):
                    tp = pps_pool.tile([128, 2, 512], F32, name="tp",
                                       tag="pps")
                    tpb = tp[:, 0].bitcast(BF16).rearrange(
                        "p (j f) -> p j f", f=128)
                    for j in range(jn):
                        nc.tensor.transpose(tpb[:, j], srcs[j0 + j], ident)
                    nc.scalar.copy(
                        out=lhst[:, b, j0:j0 + jn, c * 128:(c + 1) * 128],
                        in_=tpb[:, 0:jn])

        # --- persistent padded binarized buffers (pads zeroed once) --------
        bpads = [bpad_pool.tile([128, CI_BLKS, IMG_FA], BF16,
                                name=f"bpad{i}", tag=f"bpad{i}")
                 for i in range(2)]
        for bp in bpads:
            for b in range(CI_BLKS):
                nc.vector.memset(bp[:, b, 0:60], 0.0)
                nc.vector.memset(bp[:, b, 1 + 57 * WP:IMG_FA], 0.0)
                nc.vector.memset(
                    bp[:, b, 58:58 + 57 * WP].rearrange(
                        "p (h w) -> p h w", w=WP)[:, :, 0:2],
                    0.0,
                )
        # V fp8 buffers: guard tile + alignment pad zeroed once
        vfps = [vfp_pool.tile([128, CI_BLKS, 4, VPAD], FP8,
                              name=f"vfp{i}", tag=f"vfp{i}")
                for i in range(2)]
        for vf in vfps:
            for b in range(CI_BLKS):
                nc.vector.memset(vf[:, b, :, NT * VROW:VPAD], 0.0)

        # --- per-image prep pieces ----------------------------------------
        def binz_x(xs, bp, ri, b):
            # binarize to +-1 bf16 on the SCALAR engine (Sign activation);
            # data rows q=r+1, cols 1..56
            r0, r1 = XR[ri]
            dst = bp[:, b, 60:60 + H * WP].rearrange(
                "p (h w) -> p h w", w=WP)[:, r0:r1, 0:W]
            src = xs[:, (r0 % 28) * W:(r0 % 28 + (r1 - r0)) * W].rearrange(
                "p (h w) -> p h w", w=W)
            nc.scalar.activation(out=dst, in_=src, func=AF.Sign, scale=1.0)

        def v_half(bp, vf, b, hh):
            # V taps for row tiles [i0, i1), written straight to fp8
            i0, i1 = (0, 13) if hh == 0 else (13, NT)
            rows = bp[:, b, 1:1 + 58 * WP].rearrange(
                "p (i two c) -> p i two c", two=2, c=WP)
            ev, od = rows[:, :, 0], rows[:, :, 1]
            nt = i1 - i0
            d0 = ev[:, i0:i0 + nt]
            d1 = od[:, i0:i0 + nt]
            d2 = ev[:, i0 + 1:i0 + 1 + nt]
            d3 = od[:, i0 + 1:i0 + 1 + nt]
            vbt = vf[:, b, :, 0:29 * VROW].rearrange(
                "p t (i c) -> p t i c", c=VROW)
            nc.vector.tensor_tensor(vbt[:, 0, i0:i1], d0, d2, ALU.subtract)
            nc.vector.tensor_tensor(vbt[:, 1, i0:i1], d1, d2, ALU.add)
            nc.vector.tensor_tensor(vbt[:, 2, i0:i1], d2, d1, ALU.subtract)
            nc.vector.tensor_tensor(vbt[:, 3, i0:i1], d3, d1, ALU.subtract)

        # --- conv groups ---------------------------------------------------
        bias_sb = consts.tile([128, CO_BLKS], F32)

        def conv_group(n, vf, c, ks):
            # ks: 1 or 2 chunk indices sharing one P and one Q psum tile
            pp = pps_pool.tile([128, 2, 512], F32, name="pp", tag="pps")
            qq = qps_pool.tile([128, 2, 512], F32, name="qq", tag="qps")
            for ki, k in enumerate(ks):
                for t, (dst, ti) in enumerate(((qq, 0), (pp, 0),
                                               (pp, 1), (qq, 1))):
                    for kw in range(3):
                        off = 4 * k * VROW + kw
                        nc.tensor.matmul(
                            dst[:, ki, ti * NW:(ti + 1) * NW],
                            lhst[:, :, t * 3 + kw, c * 128:(c + 1) * 128],
                            vf[:, 0:2, t, off:off + NW],
                            start=(kw == 0), stop=(kw == 2),
                            perf_mode=DR,
                        )
            nk = len(ks)
            ppr = pp[:, :, 0:2 * NW].rearrange("p k (t f) -> p k t f", f=NW)
            qqr = qq[:, :, 0:2 * NW].rearrange("p k (t f) -> p k t f", f=NW)
            P = p_pool.tile([128, 2, 2, NW], BF16, name="P")
            nc.scalar.activation(out=P[:, 0:nk], in_=ppr[:, 0:nk],
                                 func=AF.Identity, scale=2.0)
            Q = q_pool.tile([128, 2, 2, NW], BF16, name="Q")
            nc.scalar.activation(out=Q[:, 0:nk], in_=qqr[:, 0:nk],
                                 func=AF.Identity,
                                 bias=bias_sb[:, c:c + 1], scale=2.0)
            for ki, k in enumerate(ks):
                uv = uv_pool.tile([128, 2, NW], BF16, name="uv")
                nc.vector.tensor_tensor(uv[:, 0], P[:, ki, 0], P[:, ki, 1],
                                        ALU.add)
                nc.vector.tensor_tensor(uv[:, 1], P[:, ki, 0], P[:, ki, 1],
                                        ALU.subtract)
                osb = out_pool.tile([128, R, W], F32, name="osb")
                osbr = osb.rearrange("p (i two) w -> p i two w", two=2)
                uvr = uv.rearrange("p t (i c) -> p t i c", c=VROW)
                Qr = Q.rearrange("p k t (i c) -> p k t i c", c=VROW)
                nc.vector.tensor_tensor(osbr[:, :, 0], uvr[:, 0, :, 0:W],
                                        Qr[:, ki, 0, :, 0:W], ALU.add)
                nc.vector.tensor_tensor(osbr[:, :, 1], uvr[:, 1, :, 0:W],
                                        Qr[:, ki, 1, :, 0:W], ALU.add)
                nc.sync.dma_start(
                    out=y_ap[n, c * 128:(c + 1) * 128]
                        .rearrange("co h w -> co (h w)")
                        [:, R * W * k:R * W * (k + 1)],
                    in_=osb.rearrange("p r w -> p (r w)"),
                )

        # prep pieces for image n: 8 callables interleaved with conv groups
        def prep_pieces(n):
            bp = bpads[n % 2]
            vf = vfps[n % 2]
            xs = [[xstage_pool.tile([128, 28 * W], F32, name=f"xs{n}{hb}{b}",
                                    tag="xs")
                   for b in range(2)] for hb in range(2)]
            for hb in range(2):
                for b in range(2):
                    dma_x(xs[hb][b], n, 2 * hb, b)
                    dma_x(xs[hb][b], n, 2 * hb + 1, b)
            def piece(i):
                if i == 0:
                    binz_x(xs[0][0], bp, 0, 0)
                    binz_x(xs[0][0], bp, 1, 0)
                elif i == 1:
                    binz_x(xs[0][1], bp, 0, 1)
                    binz_x(xs[0][1], bp, 1, 1)
                elif i == 2:
                    v_half(bp, vf, 0, 0)
                elif i == 3:
                    v_half(bp, vf, 1, 0)
                elif i == 4:
                    binz_x(xs[1][0], bp, 2, 0)
                    binz_x(xs[1][0], bp, 3, 0)
                elif i == 5:
                    binz_x(xs[1][1], bp, 2, 1)
                    binz_x(xs[1][1], bp, 3, 1)
                elif i == 6:
                    v_half(bp, vf, 0, 1)
                elif i == 7:
                    v_half(bp, vf, 1, 1)
            return piece

        # --- emission ------------------------------------------------------
        dma_w(0, 0, wstage)
        dma_w(0, 1, wstage)
        p0 = prep_pieces(0)
        nc.scalar.dma_start(out=bias_sb,
                            in_=b_ap.rearrange("(b p) -> p b", p=128))
        wstage2 = wstage_pool.tile([128, CIN, 9], F32, name="wstage2",
                                   tag="wstage")
        dma_w(1, 0, wstage2)
        dma_w(1, 1, wstage2)
        wprep(0, wstage)
        for i in range(4):
            p0(i)
        wprep(1, wstage2)
        for _ in range(10):
            junk_mm()
        # img0 first half conv (k 0..2, row tiles <= 12), k-major, with the
        # second-half prep pieces interleaved
        vf0 = vfps[0]
        for gi, ks in enumerate(((0, 1), (2,))):
            for c in range(CO_BLKS):
                conv_group(0, vf0, c, ks)
            p0(4 + 2 * gi)
            p0(5 + 2 * gi)
        p1 = prep_pieces(1) if n_imgs > 1 else None
        gi = 0
        for ks in ((3, 4), (5, 6)):
            for c in range(CO_BLKS):
                conv_group(0, vf0, c, ks)
                if p1 is not None:
                    p1(gi * 2)
                    p1(gi * 2 + 1)
                    gi += 1
        for n in range(1, n_imgs):
            pn = prep_pieces(n + 1) if n + 1 < n_imgs else None
            vf = vfps[n % 2]
            gi = 0
            for c in range(CO_BLKS):
                for ks in ((0, 1), (2, 3), (4, 5), (6,)):
                    conv_group(n, vf, c, ks)
                    if pn is not None:
                        pn(gi)
                        gi += 1


_NC_CACHE = {}


def _get_nc(n_imgs):
    if n_imgs not in _NC_CACHE:
        nc = bacc.Bacc("TRN2", target_bir_lowering=False, debug=False)
        x_ap = nc.dram_tensor("x", [n_imgs, CIN, H, W], F32,
                              kind="ExternalInput").ap()
        w_ap = nc.dram_tensor("weight", [COUT, CIN, 3, 3], F32,
                              kind="ExternalInput").ap()
        b_ap = nc.dram_tensor("bias", [COUT], F32, kind="ExternalInput").ap()
        y_ap = nc.dram_tensor("y", [n_imgs, COUT, H, W], F32,
                              kind="ExternalOutput").ap()
        import os
        build = (_build_wino if os.environ.get("KERNEL_IMPL") == "wino"
                 else _build_conv)
        with tile.TileContext(nc) as tc:
            build(tc, y_ap, x_ap, w_ap, b_ap, n_imgs)
        nc.compile()
        _NC_CACHE[n_imgs] = nc
    return _NC_CACHE[n_imgs]


def kernel(x: np.ndarray, weight: np.ndarray, bias: np.ndarray) -> np.ndarray:
    assert x.shape[1:] == (CIN, H, W), x.shape
    assert x.shape[0] % N_CORES == 0, x.shape
    n_imgs = x.shape[0] // N_CORES
    x = np.ascontiguousarray(x, dtype=np.float32)
    weight = np.ascontiguousarray(weight, dtype=np.float32)
    bias = np.ascontiguousarray(bias, dtype=np.float32)

    nc = _get_nc(n_imgs)
    shards = [x[i * n_imgs:(i + 1) * n_imgs] for i in range(N_CORES)]
    in_maps = [{"x": s, "weight": weight, "bias": bias} for s in shards]
    res = run_bass_kernel_spmd(nc, in_maps, core_ids=list(range(N_CORES)))
    return np.concatenate([r["y"] for r in res.results], axis=0)

